# revision 1
# baseline (speedup 1.0000x reference)
"""Trainium2 Bass kernel for the pickup/delivery heterogeneous MHA module.

Shapes (hardcoded): q (16, 501, 128) f32, 8 heads, key dim 16,
n_pick = n_delivery = 250, G = 1 + 250 + 250 = 501.

Sharding: data parallel over batch — 2 batches per core on 8 cores.

v2 layout: scores and attn@V are packed 4-heads x 32-keys per 128-partition
PSUM tile via block-diagonal weights, so every streamed column does
full-width work in a single matmul (no concurrent-pair XBUS sharing):

  score MM:  out[(h4,k32), q] = bdk_q[128(h4,kd32p), 128(h4,k32)].T @ QT_q
  attn@V MM: H[(h4, 16V+16den), q] += bdv_q[128(h4,k32), 128].T @ ex

Gapped axis (rotated + padded): picks at 0..250, zeros 250..256, delivs at
256..506, depot 506, zeros 507..512. Key groups are then uniform 32-wide
slices g_j = [32j, 32j+32), j = 0..15; the same bdk serves the main scores
(vs QT) and the block scores (vs QP/QD). Contraction rows are per-quad
(h' = head%4) at 32h'+kd with zero padding rows 32h'+16..32h'+32, so all
spread copies are 32-partition-aligned. Validity (gap keys, junk tails,
depot exclusion for block attention) is enforced on the V side: bdv zero
rows + ones-column denominator patterns.
"""

import sys

for _p in ("/opt/trn_rl_repo", "/root/.axon_site/_ro/trn_rl_repo"):
    if _p not in sys.path:
        sys.path.insert(0, _p)

import math

import ml_dtypes
import numpy as np

B, G, D, H, KD = 16, 501, 128, 8, 16
NP = ND = 250
NCORES = 8
BPC = B // NCORES  # batches per core

G2 = 512          # gapped/padded axis length
GQ = 507          # gapped query count (picks 250 + gap 6 + delivs 250 + depot)
GB = 506          # block query count (no depot)

# rotated g order: [picks (g 1..251), delivs (g 251..501), depot (g 0)]
ROT = np.concatenate([np.arange(1, G), [0]])

# V-projection key chunks (keys on partitions, 128-aligned with the groups)
CHUNKS_V = [(0, 128), (128, 256), (256, 384), (384, 507)]
# final-out chunks: (col0, col1, out_row_start, rows_used)
CHUNKS_O = [(0, 128, 1, 128), (128, 256, 129, 122),
            (256, 384, 251, 128), (384, 507, 379, 122)]
# per-group valid key rows: (main, block)
GROUP_NV = [(32, 32)] * 7 + [(26, 26)] + [(32, 32)] * 7 + [(27, 26)]

# units whose exp runs on the vector engine (Schraudolph bit trick, ~3% per
# element) instead of the scalar engine; placed in slots where the DVE queue
# is otherwise idle (after the bdv spreads, before the quad-0 tail)
DVE_UNITS = frozenset({4, 7, 10, 13})

_CACHE = {}


def _build_nc(loop_k=0, dve_units=None):
    if dve_units is None:
        dve_units = DVE_UNITS
    """loop_k=0: normal kernel. loop_k>0: wrap the body in a device-side
    For_i loop of loop_k iterations (benchmarking only).

    dve_units: unit indices (0..31, unit = 2*group+quad) whose exp runs on
    the vector engine (Schraudolph bit trick) instead of the scalar engine."""
    import contextlib

    import concourse.bacc as bacc
    import concourse.mybir as mybir
    import concourse.tile as tile

    f32 = mybir.dt.float32
    bf16 = mybir.dt.bfloat16
    i32 = mybir.dt.int32
    i16 = mybir.dt.int16
    EXP = mybir.ActivationFunctionType.Exp
    MULT = mybir.AluOpType.mult
    ADD = mybir.AluOpType.add
    MAX = mybir.AluOpType.max

    # Schraudolph: i = A*s + B as int32; top 16 bits are a bf16 ~ exp(s)
    SCH_A = float(2 ** 23 / math.log(2.0))
    SCH_B = float(127.0 * 2 ** 23 - 366393.0)

    nc = bacc.Bacc("TRN2", target_bir_lowering=False, debug=False,
                   num_devices=NCORES)

    hq = nc.dram_tensor("hq", [BPC, 2, D, G2], bf16, kind="ExternalInput")
    # bf16 weights packed into one DMA; per-quad head-padded stacks
    wnames = ["wq0", "wq1", "wk0", "wk1", "w1q0", "w1q1", "w2q0", "w2q1",
              "w3q0", "w3q1", "w4q0", "w4q1", "wv"]
    walldr = nc.dram_tensor("wall", [D, len(wnames) * D], bf16,
                            kind="ExternalInput")
    wobfdr = nc.dram_tensor("wobf", [D, 2 * D], bf16, kind="ExternalInput")
    seldr = nc.dram_tensor("sel", [D, D], bf16, kind="ExternalInput")
    out = nc.dram_tensor("out", [BPC, G, D], f32, kind="ExternalOutput")

    with tile.TileContext(nc) as tc:
        with (
            tc.tile_pool(name="const", bufs=1) as constp,
            tc.tile_pool(name="perb", bufs=2) as perb,
            tc.tile_pool(name="expp", bufs=6) as expp,
            tc.tile_pool(name="i32p", bufs=2) as i32p,
            tc.tile_pool(name="tmp", bufs=3) as tmpp,
            tc.tile_pool(name="outp", bufs=3) as outp,
            tc.tile_pool(name="ps", bufs=2, space="PSUM") as psp,
            tc.tile_pool(name="hps", bufs=2, space="PSUM") as hpsp,
        ):
            wall = constp.tile([D, len(wnames) * D], bf16, name="wall")
            nc.sync.dma_start(wall[:], walldr.ap())
            wsb = {n: wall[:, i * D:(i + 1) * D]
                   for i, n in enumerate(wnames)}
            sel_sb = constp.tile([D, D], bf16, name="sel_sb")
            nc.gpsimd.dma_start(sel_sb[:], seldr.ap())
            wobf = constp.tile([D, 2 * D], bf16, name="wobf")
            nc.gpsimd.dma_start(wobf[:], wobfdr.ap())
            wo_sb = (wobf[:, 0:D], wobf[:, D:2 * D])

            # block-diagonal weight tiles: [quad][batch-parity]
            bdk = [[constp.tile([D, 16, 128], bf16, name=f"bdk{q}_{i}")
                    for i in range(2)] for q in (0, 1)]
            bdv = [[constp.tile([D, 16, 128], bf16, name=f"bdv{q}_{i}")
                    for i in range(2)] for q in (0, 1)]
            bdvb15 = [[constp.tile([D, 128], bf16, name=f"bdvb15_{q}_{i}")
                       for i in range(2)] for q in (0, 1)]
            # zero background, set once; per-batch spreads only rewrite the
            # data blocks so everything else stays zero across iterations
            for q in (0, 1):
                for i in range(2):
                    nc.gpsimd.memset(bdk[q][i][:], 0.0)
                    nc.gpsimd.memset(bdv[q][i][:], 0.0)
                    nc.gpsimd.memset(bdvb15[q][i][:], 0.0)
            # ones columns (softmax denominator accumulators), constant
            for q in (0, 1):
                for i in range(2):
                    for j in range(16):
                        nvm = GROUP_NV[j][0]
                        for hh in range(4):
                            p0 = 32 * hh
                            nc.gpsimd.memset(
                                bdv[q][i][p0:p0 + nvm, j,
                                          p0 + 16:p0 + 32], 1.0)
                    for hh in range(4):
                        p0 = 32 * hh
                        nc.gpsimd.memset(
                            bdvb15[q][i][p0:p0 + 26, p0 + 16:p0 + 32], 1.0)

            # per-quad K tiles; tail cols 507..512 must be zero (read by the
            # bdk spreads into g15 cols 27..32)
            kt_tiles = [[constp.tile([D, G2], bf16, name=f"kt{q}_{i}")
                         for i in range(2)] for q in (0, 1)]
            for q in (0, 1):
                for i in range(2):
                    nc.gpsimd.memset(kt_tiles[q][i][:, 500:512], 0.0)

            # warm the ACT exp table; dummy matmuls release the HAM gate
            wtile = constp.tile([1, 4], f32, name="wtile")
            nc.gpsimd.memset(wtile[:], 0.0)
            wtile2 = constp.tile([1, 4], f32, name="wtile2")
            nc.scalar.activation(wtile2[:], wtile[:], EXP)
            wz = constp.tile([D, 64], bf16, name="wz")
            nc.gpsimd.memset(wz[:], 0.0)

            def proj_dma(b, st):
                hT = perb.tile([D, 2, G2], bf16, name="hT")
                src = hq.ap()[b].transpose([1, 0, 2])
                nc.sync.dma_start(hT[:, 0, :], src[:, 0, :])
                nc.sync.dma_start(hT[:, 1, :], src[:, 1, :])
                st["hT"] = hT

            def pmm(dst, w, st, col0, col1, first, last=True):
                hT = st["hT"]
                nc.tensor.matmul(dst, w, hT[:, 0, col0:col1],
                                 start=first, stop=False,
                                 skip_group_check=True)
                nc.tensor.matmul(dst, w, hT[:, 1, col0:col1],
                                 start=False, stop=last,
                                 skip_group_check=True)

            def proj_qt(b, st, q):
                qt_ps = psp.tile([128, 512], f32, tag="aux", bufs=2,
                                 name="qt_ps")
                pmm(qt_ps[:, 0:GQ], wsb[f"wq{q}"], st, 0, GQ, True)
                qt = perb.tile([D, G2], bf16, name=f"qt{q}")
                nc.vector.tensor_copy(qt[:, 0:GQ], qt_ps[:, 0:GQ])
                st.setdefault("qt", {})[q] = qt

            def proj_kt(b, st, q):
                kt_ps = psp.tile([128, 512], f32, tag="aux", bufs=2,
                                 name="kt_ps")
                pmm(kt_ps[:, 0:GQ], wsb[f"wk{q}"], st, 0, GQ, True)
                kt = kt_tiles[q][b % 2]
                nc.vector.tensor_copy(kt[:, 0:GQ], kt_ps[:, 0:GQ])
                st.setdefault("kt", {})[q] = kt

            def bdk_spread(b, st, q):
                """4 DMA moves: kt_q head bands -> block-diagonal slots.
                SBUF->SBUF byte moves on the DMA engines keep the DVE free."""
                kt = st["kt"][q]
                dst = bdk[q][b % 2]
                st.setdefault("bdk", {})[q] = dst
                for hh in range(4):
                    p0 = 32 * hh
                    src = kt[p0:p0 + 16, :].rearrange(
                        "p (j k) -> p j k", j=16)
                    nc.sync.dma_start(
                        dst[p0:p0 + 16, :, p0:p0 + 32], src)

            def proj_v(b, st, half):
                hT = st["hT"]
                if half == 0:
                    st["v_ps"] = psp.tile([128, 512], f32, tag="aux",
                                          bufs=2, name="v_ps")
                v_ps = st["v_ps"]
                for ci in (2 * half, 2 * half + 1):
                    c0, c1 = CHUNKS_V[ci]
                    for i in range(2):
                        nc.tensor.matmul(
                            v_ps[0:c1 - c0, 128 * ci:128 * ci + 128],
                            hT[:, i, c0:c1], wsb["wv"],
                            start=(ci == 0 and i == 0),
                            stop=(ci == 3 and i == 1),
                            skip_group_check=True)

            def v_copy(b, st):
                v_sb = perb.tile([128, 512], bf16, name="v_sb")
                nc.vector.tensor_copy(v_sb[:], st["v_ps"][:])
                st["v_sb"] = v_sb

            def bdv_spread(b, st, a):
                """V -> bdv block positions for slab a (groups j = 4c+a)."""
                v_sb = st["v_sb"]
                for q in (0, 1):
                    dstt = bdv[q][b % 2]
                    dst4 = dstt.rearrange("p (c a) w -> p c a w", a=4)
                    for hh in range(4):
                        p0 = 32 * hh
                        sc0 = 64 * q + 16 * hh
                        src = v_sb[32 * a:32 * a + 32, :].rearrange(
                            "p (c w) -> p c w", c=4)[:, :, sc0:sc0 + 16]
                        if a < 3:
                            nc.sync.dma_start(
                                dst4[p0:p0 + 32, :, a, p0:p0 + 16], src)
                        else:
                            # c=3 is g15: only 27 valid key rows
                            nc.sync.dma_start(
                                dst4[p0:p0 + 32, 0:3, 3, p0:p0 + 16],
                                src[:, 0:3, :])
                            nc.sync.dma_start(
                                dst4[p0:p0 + 27, 3, 3, p0:p0 + 16],
                                src[0:27, 3, :])
                st["bdv"] = [bdv[0][b % 2], bdv[1][b % 2]]

            def bdvb15_spread(b, st):
                """block-attention variant of g15: excludes the depot
                (row 26). rows 0..26 = keys 480..506 = chunk3 parts 96..122."""
                v_sb = st["v_sb"]
                for q in (0, 1):
                    dstt = bdvb15[q][b % 2]
                    for hh in range(4):
                        p0 = 32 * hh
                        sc0 = 64 * q + 16 * hh
                        nc.sync.dma_start(
                            dstt[p0:p0 + 26, p0:p0 + 16],
                            v_sb[96:122, 384 + sc0:384 + sc0 + 16])
                st["bdvb15"] = [bdvb15[0][b % 2], bdvb15[1][b % 2]]

            def proj_qp(b, st, q):
                qp_ps = psp.tile([128, 512], f32, tag="aux", bufs=2,
                                 name="qp_ps")
                pmm(qp_ps[:, 0:250], wsb[f"w1q{q}"], st, 0, 250, True,
                    last=False)
                # w3 range starts at the gap: cols 250..256 become zeros
                pmm(qp_ps[:, 250:GB], wsb[f"w3q{q}"], st, 250, GB, False)
                qp = perb.tile([D, G2], bf16, name=f"qp{q}")
                nc.vector.tensor_copy(qp[:, 0:GB], qp_ps[:, 0:GB])
                st.setdefault("qp", {})[q] = qp

            def proj_qd(b, st, q):
                qd_ps = psp.tile([128, 512], f32, tag="aux", bufs=2,
                                 name="qd_ps")
                pmm(qd_ps[:, 0:250], wsb[f"w2q{q}"], st, 0, 250, True,
                    last=False)
                pmm(qd_ps[:, 250:GB], wsb[f"w4q{q}"], st, 250, GB, False)
                qd = perb.tile([D, G2], bf16, name=f"qd{q}")
                nc.vector.tensor_copy(qd[:, 0:GB], qd_ps[:, 0:GB])
                st.setdefault("qd", {})[q] = qd

            def stream_units(b, st):
                """32 units: unit u = (group j=u//2, quad q=u%2):
                2 score MMs (main vs QT, block vs QP/QD) -> exp ->
                2 attn@V MMs accumulating into H_q."""
                H0 = hpsp.tile([128, 512], f32, tag="H", name="H0")
                H1 = hpsp.tile([128, 512], f32, tag="H", name="H1")
                st["Hs"] = (H0, H1)

                def unit(u):
                    # quad 0 first (units 0..15) so its tail can start at
                    # slot 18 while quad 1 streams
                    j, q = u % 16, u // 16
                    pend = []

                    def emit_scores():
                        cur = psp.tile([128, 2, 512], f32, tag="sc",
                                       name="sc")
                        bdk_q = st["bdk"][q][:, j, :]
                        nc.tensor.matmul(cur[:, 0, 0:GQ], bdk_q,
                                         st["qt"][q][:, 0:GQ])
                        qb = st["qp"] if j < 8 else st["qd"]
                        nc.tensor.matmul(cur[:, 1, 0:GB], bdk_q,
                                         qb[q][:, 0:GB])
                        ex = expp.tile([128, 2, 512], bf16, name="ex")
                        if u in dve_units:
                            # Schraudolph exp: DVE makes the int32 pattern
                            # from PSUM, GpSimd extracts the bf16 top bits
                            sci = i32p.tile([128, 2, 512], i32, name="sci")
                            nc.vector.tensor_scalar(
                                sci[:, :, 0:GQ], cur[:, :, 0:GQ],
                                SCH_A, SCH_B, MULT, ADD)
                            nc.vector.tensor_scalar(
                                ex[:, :, 0:GQ].bitcast(i16),
                                sci[:, :, 0:GQ],
                                float(1.0 / 65536.0), 1.0, MULT, MAX)
                        else:
                            nc.scalar.activation(ex[:, :, 0:GQ],
                                                 cur[:, :, 0:GQ], EXP)
                        pend.append(ex)

                    def emit_avs():
                        ex = pend.pop()
                        Hq = st["Hs"][q]
                        bdv_q = st["bdv"][q][:, j, :]
                        nc.tensor.matmul(
                            Hq[:, 0:GQ], bdv_q, ex[:, 0, 0:GQ],
                            start=(j == 0), stop=False,
                            skip_group_check=True)
                        bdvb_q = (st["bdvb15"][q] if j == 15 else bdv_q)
                        nc.tensor.matmul(
                            Hq[:, 0:GB], bdvb_q, ex[:, 1, 0:GB],
                            start=False, stop=(j == 15),
                            skip_group_check=True)
                    return emit_scores, emit_avs

                return [unit(u) for u in range(32)]

            def tail_norm_q(b, st, quad):
                """normalize one quad by its softmax denominators via the
                sel-matmul denominator broadcast; split to pipeline."""
                Hq = st["Hs"][quad]
                hsb = tmpp.tile([D, GQ], bf16, tag="hsb", name="hsb")
                denb = psp.tile([128, 512], f32, tag="aux", bufs=2,
                                name="denb")
                rcb = tmpp.tile([D, GQ], f32, tag="rcb", name="rcb")
                hn = perb.tile([D, GQ], bf16, name=f"hn{quad}")
                st[f"hn{quad}"] = hn
                for c0, c1 in ((0, 256), (256, GQ)):
                    nc.vector.tensor_copy(hsb[:, c0:c1], Hq[:, c0:c1])
                    nc.tensor.matmul(denb[:, c0:c1], sel_sb,
                                     hsb[:, c0:c1], start=(c0 == 0),
                                     stop=(c0 != 0), skip_group_check=True)
                    nc.vector.reciprocal_approx_fast(rcb[:, c0:c1],
                                                     denb[:, c0:c1])
                    nc.vector.tensor_mul(hn[:, c0:c1], hsb[:, c0:c1],
                                         rcb[:, c0:c1])
                    yield

            def run_gen(gen):
                for _ in gen:
                    pass

            def tail_final_mms(b, st, quad, cis=(0, 1, 2, 3)):
                hn = st[f"hn{quad}"]
                wo = wo_sb[quad]
                if "ops" not in st:
                    st["ops"] = [
                        psp.tile([128, 512], f32, tag="aux", bufs=2,
                                 name="ops_a"),
                        psp.tile([128, 512], f32, tag="aux", bufs=2,
                                 name="ops_b")]
                for ci in cis:
                    c0, c1 = CHUNKS_O[ci][0], CHUNKS_O[ci][1]
                    nc.tensor.matmul(
                        st["ops"][ci // 2][0:c1 - c0,
                                           128 * (ci % 2):128 * (ci % 2) + 128],
                        hn[:, c0:c1], wo,
                        start=(quad == 0 and ci % 2 == 0),
                        stop=(quad == 1 and ci % 2 == 1),
                        skip_group_check=True)

            def tail_final_out(b, st):
                osb = outp.tile([128, 2, 512], f32, name="osb")
                for ci, (c0, c1, r0, nr) in enumerate(CHUNKS_O):
                    cs = c1 - c0
                    sl = slice(128 * (ci % 2), 128 * (ci % 2) + 128)
                    nc.vector.tensor_copy(osb[0:cs, ci // 2, sl],
                                          st["ops"][ci // 2][0:cs, sl])
                    reg = osb[:, ci // 2, sl]
                    nc.sync.dma_start(out.ap()[b, r0:r0 + nr, :],
                                      reg[0:nr])
                    if ci == 3:
                        # depot: gapped query 506 = row 122 of this chunk
                        nc.sync.dma_start(out.ap()[b, 0:1, :],
                                          reg[122:123])

            def emit_stream(units, aux, u0=0, depth=2, dve_depth=4,
                            carry_in=(), carry=False):
                """Software-pipelined emission. DVE-exp units' attn@V MMs
                are deferred dve_depth units (H accumulation is
                order-independent) so the slower DVE exp chain is hidden.
                Entries are (due_step, av)."""
                avq = list(carry_in)
                for i, (sc, av) in enumerate(units):
                    for f in aux.get(i, ()):
                        f()
                    sc()
                    dd = dve_depth if (u0 + i) in dve_units else depth
                    avq.append((i + dd, av))
                    for e in [e for e in avq if e[0] <= i]:
                        avq.remove(e)
                        e[1]()
                if carry:
                    return [(d - len(units), a) for d, a in avq]
                for _, av in sorted(avq, key=lambda e: e[0]):
                    av()
                return []

            loop_cm = (tc.For_i(0, loop_k, 1) if loop_k
                       else contextlib.nullcontext())
            with loop_cm:
                st0, st1 = {}, {}
                proj_dma(0, st0)
                warm = psp.tile([128, 512], f32, tag="aux", bufs=2,
                                name="warm")
                for _ in range(24):
                    nc.tensor.matmul(warm[0:16, 0:64], wz[:, 0:16],
                                     wz[:, 0:64], skip_group_check=True)
                for q in (0, 1):
                    proj_qt(0, st0, q)
                    proj_kt(0, st0, q)
                    bdk_spread(0, st0, q)
                proj_v(0, st0, 0)
                proj_v(0, st0, 1)
                v_copy(0, st0)
                bdv_spread(0, st0, 0)
                proj_qp(0, st0, 0)
                proj_qp(0, st0, 1)
                proj_qd(0, st0, 0)
                proj_qd(0, st0, 1)
                u0 = stream_units(0, st0)
                carry = emit_stream(u0, {
                    0: [lambda: bdv_spread(0, st0, 1)],
                    1: [lambda: bdv_spread(0, st0, 2)],
                    2: [lambda: bdv_spread(0, st0, 3),
                        lambda: bdvb15_spread(0, st0)],
                    18: [lambda: run_gen(tail_norm_q(0, st0, 0))],
                    20: [lambda: proj_dma(1, st1)],
                    21: [lambda: proj_qt(1, st1, 0)],
                    22: [lambda: proj_qt(1, st1, 1)],
                    23: [lambda: proj_kt(1, st1, 0)],
                    24: [lambda: proj_kt(1, st1, 1),
                         lambda: bdk_spread(1, st1, 0)],
                    25: [lambda: bdk_spread(1, st1, 1),
                         lambda: proj_v(1, st1, 0)],
                    26: [lambda: proj_v(1, st1, 1)],
                    27: [lambda: v_copy(1, st1)],
                    28: [lambda: bdv_spread(1, st1, 0),
                         lambda: proj_qp(1, st1, 0)],
                    29: [lambda: bdv_spread(1, st1, 1),
                         lambda: proj_qp(1, st1, 1)],
                    30: [lambda: bdv_spread(1, st1, 2),
                         lambda: proj_qd(1, st1, 0)],
                    31: [lambda: bdv_spread(1, st1, 3),
                         lambda: bdvb15_spread(1, st1),
                         lambda: proj_qd(1, st1, 1)],
                }, carry=True)
                u1 = stream_units(1, st1)
                emit_stream(u1, {
                    2: [lambda: run_gen(tail_norm_q(0, st0, 1))],
                    4: [lambda: tail_final_mms(0, st0, 0)],
                    5: [lambda: tail_final_mms(0, st0, 1)],
                    6: [lambda: tail_final_out(0, st0)],
                    18: [lambda: run_gen(tail_norm_q(1, st1, 0))],
                }, carry_in=carry)
                gen = tail_norm_q(1, st1, 1)
                next(gen)                        # cols 0:256 (chunks 0, 1)
                tail_final_mms(1, st1, 0, (0, 1))
                tail_final_mms(1, st1, 1, (0, 1))
                run_gen(gen)                     # cols 256:507 (chunks 2, 3)
                tail_final_mms(1, st1, 0, (2, 3))
                tail_final_mms(1, st1, 1, (2, 3))
                tail_final_out(1, st1)

    nc.compile()
    return nc


def _prep_weights(W_query, W_key, W_val, W1, W2, W3, W4, W_out):
    nf = 0.25  # 1/sqrt(16), exact power of two

    def quad_stack(w, q, scale):
        """[D, 128]: cols 32h'+kd = w[4q+h', :, kd]*scale; pad cols zero."""
        w = np.asarray(w, np.float32)
        outw = np.zeros((D, D), np.float32)
        for hh in range(4):
            outw[:, 32 * hh:32 * hh + KD] = w[4 * q + hh] * scale
        return outw

    stack = lambda w: np.ascontiguousarray(
        np.asarray(w, np.float32).transpose(1, 0, 2).reshape(D, D))
    wo = np.asarray(W_out, np.float32)
    wo_pad = np.zeros((2, D, D), np.float32)
    for quad in range(2):
        for j in range(4):
            wo_pad[quad, 32 * j:32 * j + KD] = wo[quad * 4 + j]
    sel = np.zeros((D, D), np.float32)
    for p2 in range(D):
        sel[32 * (p2 // 32) + 16, p2] = 1.0
    blocks = []
    for w, scale in ((W_query, nf), (W_key, 1.0), (W1, nf), (W2, nf),
                     (W3, nf), (W4, nf)):
        for q in (0, 1):
            blocks.append(quad_stack(w, q, scale))
    # order must match wnames: wq0 wq1 wk0 wk1 w1q0 w1q1 w2q0 w2q1 w3q0 ...
    wall = np.concatenate(blocks + [stack(W_val)], axis=1)
    wobf = np.concatenate([wo_pad[0], wo_pad[1]],
                          axis=1).astype(ml_dtypes.bfloat16)
    return {"wall": np.ascontiguousarray(wall.astype(ml_dtypes.bfloat16)),
            "wobf": np.ascontiguousarray(wobf),
            "sel": np.ascontiguousarray(sel.astype(ml_dtypes.bfloat16))}


def prep_inputs(q):
    """q (B, G, D) f32 -> hq (B, 2, D, 512) bf16 hi/lo, gapped layout."""
    q = np.asarray(q, np.float32)
    hTr = np.zeros((B, D, G2), np.float32)
    qrot = q[:, ROT, :]                        # picks, delivs, depot
    hTr[:, :, 0:250] = qrot[:, 0:250, :].transpose(0, 2, 1)
    hTr[:, :, 256:507] = qrot[:, 250:501, :].transpose(0, 2, 1)
    h_hi = hTr.astype(ml_dtypes.bfloat16)
    h_lo = (hTr - h_hi.astype(np.float32)).astype(ml_dtypes.bfloat16)
    return np.ascontiguousarray(np.stack([h_hi, h_lo], axis=1))


def _numpy_fallback(q, W_query, W_key, W_val, W1, W2, W3, W4, W_out,
                    n_pick, n_delivery):
    h = np.asarray(q, np.float64)
    Bq, Gq, _ = h.shape
    nf = 1.0 / math.sqrt(KD)
    NEG = -np.inf
    proj = lambda x, W: np.einsum("bnd,hdk->hbnk", x, np.asarray(W, np.float64))
    sc = lambda Q, K: nf * np.einsum("hbqk,hbgk->hbqg", Q, K)
    zm = lambda c: np.where(c == 0, NEG, c)
    Q, K, V = proj(h, W_query), proj(h, W_key), proj(h, W_val)
    comp = sc(Q, K)
    hp, hd = h[:, 1:1 + n_pick], h[:, 1 + n_pick:]
    Kp, Vp = proj(hp, W_key), proj(hp, W_val)
    Kd, Vd = proj(hd, W_key), proj(hd, W_val)
    c_pp = zm(sc(proj(hp, W1), Kp))
    c_pd = zm(sc(proj(hp, W2), Kd))
    c_dp = zm(sc(proj(hd, W3), Kp))
    c_dd = zm(sc(proj(hd, W4), Kd))

    def place(blk, r0):
        full = np.full((H, Bq, Gq, blk.shape[3]), NEG)
        full[:, :, r0:r0 + blk.shape[2], :] = blk
        return full

    md = hd.shape[1]
    cf = np.concatenate([comp, place(c_pp, 1), place(c_pd, 1),
                         place(c_dd, Gq - md), place(c_dp, Gq - md)], axis=-1)
    cf -= cf.max(axis=-1, keepdims=True)
    e = np.exp(cf)
    attn = e / e.sum(axis=-1, keepdims=True)
    g, mp = Gq, n_pick
    heads = np.einsum("hbqg,hbgv->hbqv", attn[..., :g], V)
    heads += np.einsum("hbqp,hbpv->hbqv", attn[..., g:g + mp], Vp)
    heads += np.einsum("hbqd,hbdv->hbqv", attn[..., g + mp:g + mp + md], Vd)
    heads += np.einsum("hbqd,hbdv->hbqv",
                       attn[..., g + mp + md:g + mp + 2 * md], Vd)
    heads += np.einsum("hbqp,hbpv->hbqv", attn[..., g + mp + 2 * md:], Vp)
    return np.einsum("hbqv,hve->bqe", heads,
                     np.asarray(W_out, np.float64)).astype(np.float32)


def kernel(q, W_query, W_key, W_val, W1_query, W2_query, W3_query, W4_query,
           W_out, n_pick, n_delivery):
    np_, nd_ = int(n_pick), int(n_delivery)
    q = np.asarray(q, np.float32)
    if np_ != NP or nd_ != ND or q.shape != (B, G, D):
        return _numpy_fallback(q, W_query, W_key, W_val, W1_query, W2_query,
                               W3_query, W4_query, W_out, np_, nd_)

    from concourse import bass_utils

    if "nc" not in _CACHE:
        _CACHE["nc"] = _build_nc()
    nc = _CACHE["nc"]

    w = _prep_weights(W_query, W_key, W_val, W1_query, W2_query, W3_query,
                      W4_query, W_out)
    hsplit = prep_inputs(q)
    in_maps = [dict(w, hq=hsplit[BPC * c:BPC * (c + 1)])
               for c in range(NCORES)]
    res = bass_utils.run_bass_kernel_spmd(nc, in_maps,
                                          core_ids=list(range(NCORES)))
    return np.concatenate([r["out"] for r in res.results], axis=0)

